# revision 74
# baseline (speedup 1.0000x reference)
"""Masked multi-head attention on 8 Trainium2 NeuronCores.

Sharding: batch x head-group. Core c handles batch c//4 and heads
4*(c%4) .. 4*(c%4)+3 (Wq/Wk/Wv column-sharded, Wo row-sharded). Each core
computes a partial [S, D_MODEL] output = attn_heads @ Wo_slice; the host sums
the 4 partials per batch (the row-parallel reduce) and adds bo + bv @ Wo
(the bv term folds out because softmax rows sum to 1).

Device kernel (per core), all matmuls in bfloat16 (full PE rate at any
moving width, half DMA bytes):
  per 512-wide s block j: q/k projected transposed [dout, s], v natural
  [s, dout] with a fused ones-column for softmax row sums; scores [sk, sq]
  per head with exp on the Activation engine over pair-merged 2-bank psum
  tiles; AV flipped (out [sq, d], et chunks stationary, v moving 65-wide)
  so causality halves the charged PE columns; per-row 1/sum division on
  DVE; two heads' outputs transposed back to [d, sq] in one PE transpose;
  output projection from resident oT/Wo tiles; y stored bf16.

Because every engine queue executes in order, next-block projections and
previous-block output projections are queued as small "filler" units and
popped between score pairs: the PE stays busy during exp latency without
ever head-blocking the next score matmul that feeds the Activation engine.
All DMAs are merged (halves per weight/activation block, one per output
row-tile) to amortize the per-descriptor-generation overhead; output
stores go through the gpsimd SWDGE (or SP once input loads are done) so
they never head-block input prefetches.
"""

from collections import deque

import numpy as np

D_MODEL = 1024
N_HEAD = 16
HEAD_DIM = 64
B, S = 2, 2048
GH = 4  # heads per core
GC = GH * HEAD_DIM  # 256 dout columns per core
SBK = 512  # s block (moving free dim)
NSB = S // SBK  # 4 s blocks
NKT = D_MODEL // 128  # 8 din tiles
NST = S // 128  # 16 sk tiles

_CACHE = {}


def _build_nc():
    import concourse.mybir as mybir
    from concourse import bacc, tile

    F32 = mybir.dt.float32
    BF16 = mybir.dt.bfloat16
    EXP = mybir.ActivationFunctionType.Exp

    nc = bacc.Bacc(None, target_bir_lowering=False)

    xq = nc.declare_dram_parameter("xq", [128, NKT, S], BF16, isOutput=False)
    xk = nc.declare_dram_parameter("xk", [128, NKT, S], BF16, isOutput=False)
    xv = nc.declare_dram_parameter("xv", [128, NKT, S], BF16, isOutput=False)
    wq = nc.declare_dram_parameter("wq", [128, NKT, GC], BF16, isOutput=False)
    wk = nc.declare_dram_parameter("wk", [128, NKT, GC], BF16, isOutput=False)
    wv = nc.declare_dram_parameter("wv", [128, NKT, GC], BF16, isOutput=False)
    wo = nc.declare_dram_parameter("wo", [128, 2, D_MODEL], BF16, isOutput=False)
    bq = nc.declare_dram_parameter("bq", [128, 2], F32, isOutput=False)
    bk = nc.declare_dram_parameter("bk", [128, 2], F32, isOutput=False)
    y = nc.declare_dram_parameter("y", [128, NST, D_MODEL], BF16, isOutput=True)

    with tile.TileContext(nc) as tc:
        with (
            tc.tile_pool(name="res", bufs=1) as res,
            tc.tile_pool(name="work", bufs=2) as work,
            tc.tile_pool(name="xin", bufs=1) as xin,
            tc.tile_pool(name="bigps", bufs=2, space="PSUM") as bigps,
            tc.tile_pool(name="medps", bufs=1, space="PSUM") as medps,
            tc.tile_pool(name="smallps", bufs=1, space="PSUM") as smallps,
        ):
            # ---- resident weights + first block of activations ----
            wq_sb = res.tile([128, NKT, GC], BF16, tag="wq")
            wk_sb = res.tile([128, NKT, GC], BF16, tag="wk")
            wv_sb = res.tile([128, NKT, GC], BF16, tag="wv")
            wo_sb = res.tile([128, 2, D_MODEL], BF16, tag="wo")
            bq_sb = res.tile([128, 2], F32, tag="bq")
            bk_sb = res.tile([128, 2], F32, tag="bk")

            xq_t, xk_t, xv_t = {}, {}, {}

            def load_w(dst, src):
                for half in range(2):
                    nc.sync.dma_start(dst[:, half * 4 : half * 4 + 4, :],
                                      src[:, half * 4 : half * 4 + 4, :])

            def load_x(nm, tbl, src, j):
                # two half-tiles per block so compute can start on the first
                # four k-tiles while the second half is still in flight
                t = [
                    xin.tile([128, NKT // 2, SBK], BF16, tag=f"{nm}{j % 2}{half}", name=f"{nm}_{j}_{half}")
                    for half in range(2)
                ]
                for half in range(2):
                    nc.sync.dma_start(
                        t[half][:],
                        src[:, half * 4 : half * 4 + 4, j * SBK : (j + 1) * SBK],
                    )
                tbl[j] = t

            # startup order: first halves of q and k paths first so the
            # first eight projection matmuls can begin as soon as possible.
            xq_t[0] = [xin.tile([128, 4, SBK], BF16, tag=f"xq0{half}", name=f"xq_0_{half}") for half in range(2)]
            xk_t[0] = [xin.tile([128, 4, SBK], BF16, tag=f"xk0{half}", name=f"xk_0_{half}") for half in range(2)]
            nc.sync.dma_start(wq_sb[:, 0:4, :], wq[:, 0:4, :])
            nc.sync.dma_start(xq_t[0][0][:], xq[:, 0:4, 0:SBK])
            nc.sync.dma_start(wk_sb[:, 0:4, :], wk[:, 0:4, :])
            nc.sync.dma_start(xk_t[0][0][:], xk[:, 0:4, 0:SBK])
            nc.sync.dma_start(bq_sb[:], bq[:])
            nc.sync.dma_start(bk_sb[:], bk[:])
            nc.sync.dma_start(wq_sb[:, 4:8, :], wq[:, 4:8, :])
            nc.sync.dma_start(xq_t[0][1][:], xq[:, 4:8, 0:SBK])
            nc.sync.dma_start(wk_sb[:, 4:8, :], wk[:, 4:8, :])
            nc.sync.dma_start(xk_t[0][1][:], xk[:, 4:8, 0:SBK])
            load_w(wv_sb, wv)
            load_x("xv", xv_t, xv, 0)
            nc.sync.dma_start(wo_sb[:], wo[:])

            # ---- constant tiles ----
            # tri: keep col >= partition (upper-right incl. diagonal) in
            # [sk, sq] layout; ident: 1 on the diagonal.
            tri = res.tile([128, 128], BF16, tag="tri")
            nc.gpsimd.memset(tri[:], 1.0)
            nc.gpsimd.affine_select(
                out=tri[:], in_=tri[:], compare_op=mybir.AluOpType.is_ge,
                fill=0.0, base=0, pattern=[[1, 128]], channel_multiplier=-1,
            )
            ident = res.tile([128, 128], BF16, tag="ident")
            nc.gpsimd.memset(ident[:], 1.0)
            nc.gpsimd.affine_select(
                out=ident[:], in_=ident[:], compare_op=mybir.AluOpType.is_equal,
                fill=0.0, base=0, pattern=[[1, 128]], channel_multiplier=-1,
            )

            # ---- resident activations ----
            qT_sb = [[res.tile([128, SBK], BF16, tag=f"qT_{pt}_{j}", name=f"qT_{pt}_{j}") for j in range(NSB)] for pt in range(2)]
            kT_sb = [[res.tile([128, SBK], BF16, tag=f"kT_{pt}_{j}", name=f"kT_{pt}_{j}") for j in range(NSB)] for pt in range(2)]
            oT_sb = [[res.tile([128, SBK], BF16, tag=f"oT_{pt}_{j}", name=f"oT_{pt}_{j}") for j in range(NSB)] for pt in range(2)]
            # v_aug[jb]: [128, 4(i in block), GH, 65]; cols 0..63 = v, col 64 = 1
            v_aug = [res.tile([128, 4, GH, HEAD_DIM + 1], BF16, tag=f"vaug_{jb}", name=f"vaug_{jb}") for jb in range(NSB)]
            for jb in range(NSB):
                nc.gpsimd.memset(v_aug[jb][:, :, :, HEAD_DIM], 1.0)

            # ---- filler unit queue (see module docstring) ----
            filler = deque()

            def pop_filler(n):
                for _ in range(n):
                    if not filler:
                        return
                    filler.popleft()()

            def queue_proj_qk(j, w_sb, x_t, b_sb, dst, pt):
                cell = []

                def get():
                    if not cell:
                        cell.append(medps.tile([128, SBK], F32, tag="proj", name="pproj"))
                    return cell[0]

                for kt in range(NKT):
                    def u(kt=kt):
                        nc.tensor.matmul(
                            get()[:],
                            w_sb[:, kt, pt * 128 : (pt + 1) * 128],
                            x_t[kt // 4][:, kt % 4, :],
                            start=(kt == 0), stop=(kt == NKT - 1),
                        )
                    filler.append(u)

                def ub():
                    nc.vector.tensor_scalar_add(
                        dst[pt][j][:], get()[:], b_sb[:, pt : pt + 1]
                    )
                filler.append(ub)

            def queue_proj_v(j, sp):  # sp: st pair index (0 -> st 0,1; 1 -> st 2,3)
                cell = []

                def get():
                    if not cell:
                        cell.append(medps.tile([128, SBK], F32, tag="proj", name="pv"))
                    return cell[0]

                for sx in range(2):
                    st = sp * 2 + sx
                    for kt in range(NKT):
                        def u(sx=sx, st=st, kt=kt):
                            nc.tensor.matmul(
                                get()[:, sx * GC : (sx + 1) * GC],
                                xv_t[j][kt // 4][:, kt % 4, st * 128 : (st + 1) * 128],
                                wv_sb[:, kt, :],
                                start=(kt == 0), stop=(kt == NKT - 1),
                            )
                        filler.append(u)

                def uc():
                    pv3 = get()[:].rearrange("p (s h d) -> p s h d", s=2, h=GH)
                    nc.vector.tensor_copy(
                        v_aug[j][:, sp * 2 : sp * 2 + 2, :, 0:HEAD_DIM], pv3[:]
                    )
                filler.append(uc)

            def queue_proj(j):
                queue_proj_qk(j, wq_sb, xq_t[j], bq_sb, qT_sb, 0)
                queue_proj_qk(j, wk_sb, xk_t[j], bk_sb, kT_sb, 0)
                queue_proj_qk(j, wq_sb, xq_t[j], bq_sb, qT_sb, 1)
                queue_proj_qk(j, wk_sb, xk_t[j], bk_sb, kT_sb, 1)
                queue_proj_v(j, 0)
                queue_proj_v(j, 1)

            def out_proj_units(j, tt, dma_eng, tag="yp", act_copy=False, split_dma=False):
                cell = []

                def get():
                    if not cell:
                        cell.append(medps.tile([128, SBK], F32, tag=tag, name="yp"))
                    return cell[0]

                ycell = []

                def gety():
                    if not ycell:
                        ycell.append(work.tile([128, 2 * SBK], BF16, tag="y_sb", bufs=4, name="y_sb"))
                    return ycell[0]

                def u_mm(eb):
                    for pt in range(2):
                        nc.tensor.matmul(
                            get()[:],
                            oT_sb[pt][j][:, tt * 128 : (tt + 1) * 128],
                            wo_sb[:, pt, eb * SBK : (eb + 1) * SBK],
                            start=(pt == 0), stop=(pt == 1),
                        )

                def u1():
                    u_mm(0)

                def u2():
                    if act_copy:
                        nc.scalar.copy(gety()[:, 0:SBK], get()[:])
                    else:
                        nc.vector.tensor_copy(gety()[:, 0:SBK], get()[:])
                    if split_dma:
                        # tail: fire each half as soon as its copy lands,
                        # on separate DGE paths
                        dma_eng.dma_start(y[:, j * 4 + tt, 0:SBK], gety()[:, 0:SBK])
                    u_mm(1)

                def u3():
                    nc.vector.tensor_copy(gety()[:, SBK:], get()[:])
                    if split_dma:
                        # SP HWDGE: cheaper issue path than SWDGE desc-gen on
                        # the final drain chain (SP queue is empty at tail)
                        nc.sync.dma_start(y[:, j * 4 + tt, SBK:], gety()[:, SBK:])
                    else:
                        dma_eng.dma_start(y[:, j * 4 + tt, :], gety()[:])

                return [u1, u2, u3]

            def queue_out_proj(j, dma_eng):
                for tt in range(4):
                    filler.extend(out_proj_units(j, tt, dma_eng))

            def emit_sc(j, h, p):
                pt, po = h // 2, 64 * (h % 2)
                sc = bigps.tile([128, 2 * SBK], mybir.dt.float32, tag="sc", bufs=2, name="sc")
                for q in range(2):
                    i = 2 * p + q
                    m = i - 4 * j
                    c0 = 128 * m if m > 0 else 0
                    nc.tensor.matmul(
                        sc[:, q * SBK + c0 : (q + 1) * SBK],
                        kT_sb[pt][i // 4][po : po + 64, (i % 4) * 128 : (i % 4 + 1) * 128],
                        qT_sb[pt][j][po : po + 64, c0:],
                        start=True, stop=True,
                    )
                return sc

            def emit_exp_mask(j, p, sc):
                # bufs=12: all pairs of a head stay alive for its AV chains,
                # which overlap the next head's score/exp phase
                et = work.tile([128, 2 * SBK], BF16, tag="et", bufs=16, name="et")
                i0 = 2 * p
                if i0 - 4 * j < 0:  # off-diagonal pair: one wide exp
                    nc.scalar.activation(et[:], sc[:], EXP, scale=0.125)
                else:  # diagonal pair: exact ranges, then triangle mask
                    for q in range(2):
                        m = 2 * p + q - 4 * j
                        c0 = 128 * m
                        nc.scalar.activation(
                            et[:, q * SBK + c0 : (q + 1) * SBK],
                            sc[:, q * SBK + c0 : (q + 1) * SBK],
                            EXP, scale=0.125,
                        )
                    for q in range(2):
                        m = 2 * p + q - 4 * j
                        c0 = q * SBK + 128 * m
                        nc.vector.tensor_mul(
                            et[:, c0 : c0 + 128], et[:, c0 : c0 + 128], tri[:]
                        )
                return et

            def gen_scores(j, h, state):
                # score pairs + exps for one head; sc(0) emitted immediately
                n_i = 4 * (j + 1)
                sc = emit_sc(j, h, 0)
                for p in range(n_i // 2):
                    state["ets"].append(emit_exp_mask(j, p, sc))
                    if p + 1 < n_i // 2:
                        sc = emit_sc(j, h, p + 1)
                    yield

            def gen_av(j, h, state, opair):
                # AV chains: one sequential psum accumulation group per
                # sq-tile region (interleaved groups in one bank corrupt each
                # other), then division as each region stops
                pt, po = h // 2, 64 * (h % 2)
                stream_tail = j == NSB - 1 and h == GH - 1
                tail_units = []
                ets = state["ets"]
                av4 = smallps.tile([128, 4 * 128], mybir.dt.float32, tag="av", name="av4")
                riv = work.tile([128, 4], F32, tag="riv", bufs=2, name="riv")
                oTps = None
                if h % 2 == 1:
                    oTps = smallps.tile([128, 4 * 128], BF16, tag="oT", name="oTps")
                for tt in range(4):
                    for i in range(4 * j + tt + 1):
                        nc.tensor.matmul(
                            av4[:, tt * 128 : tt * 128 + HEAD_DIM + 1],
                            ets[i // 2][:, (i % 2) * SBK + tt * 128 : (i % 2) * SBK + (tt + 1) * 128],
                            v_aug[i // 4][:, i % 4, h, :],
                            start=(i == 0), stop=(i == 4 * j + tt),
                        )
                    # division: o = av / rowsum (rowsum in col 64 per region)
                    with tc.high_priority(offset=64):
                        nc.vector.reciprocal(
                            riv[:, tt : tt + 1], av4[:, tt * 128 + 64 : tt * 128 + 65]
                        )
                        nc.vector.tensor_scalar_mul(
                            opair[:, tt, po : po + 64],
                            av4[:, tt * 128 : tt * 128 + 64],
                            riv[:, tt : tt + 1],
                        )
                    if stream_tail:
                        # stream transpose + output projection per finished
                        # sq-tile to shorten the tail; alternate the two med
                        # psum slots so chains pipeline two-wide; the output
                        # projection of tile tt is emitted one tile late so
                        # its DVE copies (which wait on PE) never delay the
                        # next division on DVE's in-order queue
                        nc.tensor.transpose(
                            oTps[:, tt * 128 : (tt + 1) * 128],
                            opair[:, tt, :], ident[:],
                        )
                        nc.vector.tensor_copy(
                            oT_sb[pt][j][:, tt * 128 : (tt + 1) * 128],
                            oTps[:, tt * 128 : (tt + 1) * 128],
                        )
                        tail_units.extend(out_proj_units(
                            j, tt, nc.sync if tt % 2 == 0 else nc.gpsimd,
                            tag="yp" if tt % 2 == 0 else "proj",
                            split_dma=True,
                        ))
                        run_n = len(tail_units) if tt == 3 else max(0, len(tail_units) - 3)
                        for u in tail_units[:run_n]:
                            u()
                        del tail_units[:run_n]
                    yield
                if h % 2 == 1 and not stream_tail:
                    # both heads of pt group done: transpose to [d, sq]
                    for tt in range(4):
                        nc.tensor.transpose(
                            oTps[:, tt * 128 : (tt + 1) * 128], opair[:, tt, :], ident[:]
                        )
                    nc.vector.tensor_copy(oT_sb[pt][j][:], oTps[:])

            def step(gen):
                if gen is None:
                    return None
                try:
                    next(gen)
                    return gen
                except StopIteration:
                    return None

            # ---- j0 projections inline (nothing else to overlap yet),
            # q/k kt-interleaved to follow the DMA arrival order; the two
            # big-psum score slots are free this early ----
            pq0 = bigps.tile([128, 2 * SBK], mybir.dt.float32, tag="sc", bufs=2, name="pq0")
            pk0 = bigps.tile([128, 2 * SBK], mybir.dt.float32, tag="sc", bufs=2, name="pk0")
            # emission matched to DMA arrival: q halves land before k halves,
            # both pt column groups of a half arrive together
            for half in range(2):
                for p, w_sb, x_t in ((pq0, wq_sb, xq_t[0]), (pk0, wk_sb, xk_t[0])):
                    for pt in range(2):
                        for kt in range(half * 4, half * 4 + 4):
                            nc.tensor.matmul(
                                p[:, pt * SBK : (pt + 1) * SBK],
                                w_sb[:, kt, pt * 128 : (pt + 1) * 128],
                                x_t[kt // 4][:, kt % 4, :],
                                start=(kt == 0), stop=(kt == NKT - 1),
                            )
            for pt in range(2):
                nc.vector.tensor_scalar_add(
                    qT_sb[pt][0][:], pq0[:, pt * SBK : (pt + 1) * SBK], bq_sb[:, pt : pt + 1]
                )
                nc.vector.tensor_scalar_add(
                    kT_sb[pt][0][:], pk0[:, pt * SBK : (pt + 1) * SBK], bk_sb[:, pt : pt + 1]
                )
            queue_proj_v(0, 0)
            queue_proj_v(0, 1)
            v0_done = []
            filler.append(lambda: v0_done.append(1))

            # software-pipelined heads: scores/exp of head h interleave with
            # the AV chains of head h-1 so the Activation engine never idles
            # at head boundaries
            agen = None
            opair = None
            for j in range(NSB):
                # flush leftovers (this block's projections) before the first
                # score matmul of the block references qT/kT/v_aug; the
                # previous head's AV chains keep draining inside the h loop
                if j > 0:
                    while filler:
                        agen = step(agen)
                        pop_filler(2)
                if j + 1 < NSB:
                    load_x("xq", xq_t, xq, j + 1)
                    load_x("xk", xk_t, xk, j + 1)
                    load_x("xv", xv_t, xv, j + 1)
                    queue_proj(j + 1)
                if j == NSB - 1:
                    # all deferred output projections land in the last
                    # block's window: attention there is Act-bound, so this
                    # is free PE filler; earlier blocks are PE-bound and
                    # shorten by exactly this much
                    for jj in range(NSB - 1):
                        queue_out_proj(jj, nc.gpsimd if jj % 2 else nc.sync)
                for h in range(GH):
                    if h % 2 == 0:
                        opair = work.tile(
                            [128, 4, 128], BF16, tag=f"opair{(h // 2) % 2}", name="opair"
                        )
                    state = {"ets": []}
                    sgen = gen_scores(j, h, state)
                    while sgen is not None or agen is not None:
                        sgen = step(sgen)
                        pop_filler(1)
                        agen = step(agen)
                        pop_filler(1)
                    if j == 0 and h == 0:
                        # v-projection units must be emitted before the first
                        # AV chain references v_aug (in-order engine queues)
                        while not v0_done:
                            pop_filler(1)
                    agen = gen_av(j, h, state, opair)
            while agen is not None:
                agen = step(agen)
                pop_filler(2)
            pop_filler(len(filler))
    nc.finalize()
    return nc


def _run_device(Q, K, V, Wq, bq, Wk, bk, Wv, Wo):
    import ml_dtypes
    from concourse.bass_utils import run_bass_kernel_spmd

    BF = ml_dtypes.bfloat16
    if "nc" not in _CACHE:
        _CACHE["nc"] = _build_nc()
    nc = _CACHE["nc"]

    def fold(a, n):  # [n*128, m] -> [128, n, m]
        return np.ascontiguousarray(a.reshape(n, 128, a.shape[1]).transpose(1, 0, 2))

    xT = {}
    for b in range(B):
        xT[("q", b)] = fold(Q[b].T.astype(BF), NKT)
        xT[("k", b)] = fold(K[b].T.astype(BF), NKT)
        xT[("v", b)] = fold(V[b].T.astype(BF), NKT)
    in_maps = []
    for c in range(8):
        b, g = c // 4, c % 4
        cs = slice(g * GC, (g + 1) * GC)
        in_maps.append(
            {
                "xq": xT[("q", b)],
                "xk": xT[("k", b)],
                "xv": xT[("v", b)],
                "wq": fold(Wq[:, cs].astype(BF), NKT),
                "wk": fold(Wk[:, cs].astype(BF), NKT),
                "wv": fold(Wv[:, cs].astype(BF), NKT),
                "wo": fold(Wo[cs, :].astype(BF), 2),
                "bq": np.ascontiguousarray(bq[cs].reshape(2, 128).T),
                "bk": np.ascontiguousarray(bk[cs].reshape(2, 128).T),
            }
        )
    res = run_bass_kernel_spmd(nc, in_maps, core_ids=list(range(8)))
    return res


def kernel(Q, K, V, mask, Wq, bq, Wk, bk, Wv, bv, Wo, bo):
    Q = np.asarray(Q, dtype=np.float32)
    K = np.asarray(K, dtype=np.float32)
    V = np.asarray(V, dtype=np.float32)
    mask = np.asarray(mask)
    Wq, Wk, Wv, Wo = (np.asarray(a, dtype=np.float32) for a in (Wq, Wk, Wv, Wo))
    bq, bk, bv, bo = (np.asarray(a, dtype=np.float32) for a in (bq, bk, bv, bo))

    causal = bool(
        np.array_equal(mask[0], np.tril(np.ones((S, S), dtype=mask.dtype)))
    )
    if not causal:
        return _numpy_fallback(Q, K, V, mask, Wq, bq, Wk, bk, Wv, bv, Wo, bo)

    res = _run_device(Q, K, V, Wq, bq, Wk, bk, Wv, Wo)
    bo_eff = bo + bv @ Wo
    out = np.empty((B, S, D_MODEL), dtype=np.float32)
    for b in range(B):
        acc = res.results[4 * b]["y"].astype(np.float32)
        for g in range(1, 4):
            acc = acc + res.results[4 * b + g]["y"].astype(np.float32)
        out[b] = acc.transpose(1, 0, 2).reshape(S, D_MODEL) + bo_eff
    return out


def _numpy_fallback(Q, K, V, mask, Wq, bq, Wk, bk, Wv, bv, Wo, bo):
    out = np.empty((B, S, D_MODEL), dtype=np.float32)
    for b in range(B):
        q = (Q[b] @ Wq + bq).reshape(S, N_HEAD, HEAD_DIM).transpose(1, 0, 2)
        k = (K[b] @ Wk + bk).reshape(S, N_HEAD, HEAD_DIM).transpose(1, 0, 2)
        v = (V[b] @ Wv + bv).reshape(S, N_HEAD, HEAD_DIM).transpose(1, 0, 2)
        mb = mask[b] if mask.shape[0] > 1 else mask[0]
        o = np.empty((N_HEAD, S, HEAD_DIM), dtype=np.float32)
        for hh in range(N_HEAD):
            s = (q[hh] @ k[hh].T) / np.sqrt(np.float32(HEAD_DIM))
            s = np.where(mb == 0, -np.inf, s)
            s = s - s.max(-1, keepdims=True)
            e = np.exp(s)
            p = e / e.sum(-1, keepdims=True)
            o[hh] = p @ v[hh]
        out[b] = o.transpose(1, 0, 2).reshape(S, D_MODEL) @ Wo + bo
    return out
